# revision 1
# baseline (speedup 1.0000x reference)
"""Trainium2 Bass kernel for nn_BCELoss_64330020159675 (segment_reduce BCE loss).

Data-parallel over batch across 8 NeuronCores:
  phase A (per core, local batch shard of 1024 rows):
    z_i = normalize(emb_i); onehot = (labels == arange(C));
    segT[d, c] = sum_b z_i[b, d] * onehot[b, c]  (PE matmuls, d-major output)
    counts[c] = sum_b onehot[b, c]               (PE matmuls, c-major columns)
  One bf16 AllReduce of [D+1, C] (segT rows 0..D-1, counts in row D).
  phase B (overlaps the collective): load emb_j pre-transposed [D, B_local],
    column norms via Square + partition_all_reduce, z_jT = emb_jT * invnorm.
  phase C: Q[c, b] = sum_d segT[d, c] * z_jT[d, b] (PE matmuls);
    d2 = Q * (-2/cnt_c) + (1 + |seg_c|^2/cnt_c^2)  -> folded into the Sqrt
    activation's per-partition scale/bias; r = sqrt(d2); sim = 2 - r;
    sum of softplus(sim) via a pairwise product tree: (1+e^sim) factors are
    multiplied across blocks on the DVE (products of 16 stay fp32-safe) so a
    SINGLE Ln+accum replaces a 16-Ln batch and only two ACT table reloads
    occur; diag term via one fused scalar_tensor_tensor per block.
  Host: loss = (sum_cores(sp_total + diag_total) - 2B) / (B*C).

Identity used: BCEWithLogits elementwise loss = softplus(sim) - match * sim,
and sum(match * sim) = 2*B - sum_b r[b, label_b].
"""
import numpy as np

import concourse.bacc as bacc
import concourse.mybir as mybir
import concourse.tile as tile
from concourse import bass_utils

B = 8192
D = 1024
C = 1024
N_CORES = 8
BL = B // N_CORES          # 1024 rows per core
P = 128                    # partitions
NB = BL // P               # 8 batch chunks per core
ND = D // P                # 8 d chunks
NCC = C // P               # 8 class chunks (partition-major)
NBF = BL // 512            # 2 batch free-dim chunks
NCF = C // 512             # 2 class free-dim chunks
NBLK = NCC * NBF           # 16 sim blocks
EPS = 1e-12

F32 = mybir.dt.float32
BF16 = mybir.dt.bfloat16
AF = mybir.ActivationFunctionType
ALU = mybir.AluOpType
AX = mybir.AxisListType

_NC_CACHE = {}

def build_nc():
    if "nc" in _NC_CACHE:
        return _NC_CACHE["nc"]
    import concourse.bass_isa as bass_isa

    nc = bacc.Bacc(
        "TRN2", target_bir_lowering=False, debug=False, num_devices=N_CORES
    )
    emb_i = nc.dram_tensor("emb_i", [BL, D], F32, kind="ExternalInput")
    emb_jT = nc.dram_tensor("emb_jT", [D, BL], F32, kind="ExternalInput")
    labels_colmat = nc.dram_tensor("labels_colmat", [P, NB], F32, kind="ExternalInput")
    label_bcast = nc.dram_tensor("label_bcast", [P, BL], F32, kind="ExternalInput")
    iota_bcast = nc.dram_tensor("iota_bcast", [P, C], F32, kind="ExternalInput")
    ccol = nc.dram_tensor("ccol", [P, NCC], F32, kind="ExternalInput")
    out_partial = nc.dram_tensor("out_partial", [1, 2], F32, kind="ExternalOutput")

    with tile.TileContext(nc) as tc:
        with (
            tc.tile_pool(name="dram", bufs=1, space="DRAM") as dram,
            tc.tile_pool(name="const", bufs=1) as constp,
            tc.tile_pool(name="zjt", bufs=1) as zjtp,
            tc.tile_pool(name="work", bufs=2) as work,
            tc.tile_pool(name="work3", bufs=3) as work3,
            tc.tile_pool(name="dump", bufs=1) as dump,
        ):
            cc_in = dram.tile([D + 1, C], BF16)
            cc_out = dram.tile([D + 1, C], BF16, addr_space="Shared")

            ones_col = constp.tile([P, 1], F32)
            nc.vector.memset(ones_col[:], 1.0)
            ones_bf = constp.tile([P, 1], BF16)
            nc.vector.memset(ones_bf[:], 1.0)
            lab_cm = constp.tile([P, NB], F32)
            nc.gpsimd.dma_start(lab_cm[:], labels_colmat[:])
            lab_bc = constp.tile([P, BL], F32)
            nc.gpsimd.dma_start(lab_bc[:], label_bcast[:])
            iota_bc = constp.tile([P, C], F32)
            nc.gpsimd.dma_start(iota_bc[:], iota_bcast[:])
            ccol_t = constp.tile([P, NCC], F32)
            nc.gpsimd.dma_start(ccol_t[:], ccol[:])

            # ---------------- phase A ----------------
            with (
                tc.tile_pool(name="phA", bufs=1) as pa,
                tc.tile_pool(name="psA", bufs=3, space="PSUM") as psA,
            ):
                z_i = [pa.tile([P, D], BF16, name=f"zi{b}") for b in range(NB)]
                oh = [pa.tile([P, C], BF16, name=f"oh{b}") for b in range(NB)]
                sq_dump = dump.tile([P, D], F32, name="sq_dump")
                # per-chunk pipelined norms; Square/Sqrt share one ACT table
                e_last = None
                for b in range(NB):
                    e = work.tile([P, D], F32, tag="embi", bufs=4)
                    # three concurrent DMA paths: SP-HWDGE, ACT-HWDGE, SWDGE
                    dma_eng = (nc.sync, nc.scalar, nc.gpsimd)[b % 3]
                    dma_eng.dma_start(e[:], emb_i[b * P : (b + 1) * P, :])
                    e_last = e
                    ss = work.tile([P, 1], F32, tag="ss")
                    nc.scalar.activation(sq_dump[:], e[:], AF.Square, accum_out=ss[:])
                    nrm = work.tile([P, 1], F32, tag="nrm")
                    nc.scalar.activation(nrm[:], ss[:], AF.Sqrt)
                    nc.vector.tensor_scalar(nrm[:], nrm[:], EPS, None, ALU.max)
                    inv = work.tile([P, 1], F32, tag="inv")
                    nc.vector.reciprocal(inv[:], nrm[:])
                    nc.vector.tensor_scalar(z_i[b][:], e[:], inv[:], None, ALU.mult)
                    nc.vector.tensor_scalar(
                        oh[b][:], iota_bc[:], lab_cm[:, b : b + 1], None, ALU.is_equal
                    )

                # re-seed ones_bf with a dependency on z_i[6] so the counts
                # matmuls run right before the seg matmuls and serve as PE
                # warm-up (issued earlier they let the PE HAM cool again)
                nc.vector.tensor_scalar(
                    ones_bf[0:1, 0:1], z_i[NB - 2][0:1, 0:1], 0.0, 1.0,
                    ALU.mult, ALU.add,
                )
                # counts as a [1, C] row: cnt[c] = sum_b onehot[b, c]
                cnt_ps = psA.tile([1, C], F32, tag="cntrow", bufs=1)
                for half in range(NCF):
                    for b in range(NB):
                        nc.tensor.matmul(
                            cnt_ps[:, half * 512 : (half + 1) * 512],
                            ones_bf[:],
                            oh[b][:, half * 512 : (half + 1) * 512],
                            start=(b == 0),
                            stop=(b == NB - 1),
                        )
                cnt_row = work.tile([1, C], BF16, tag="cntrowsb")
                nc.scalar.copy(cnt_row[:], cnt_ps[:])
                nc.sync.dma_start(cc_in[D : D + 1, :], cnt_row[:])

                # segT matmuls: out[d_chunk, c] = sum_b z_i[b, d] * onehot[b, c]
                for d in range(ND):
                    for cf in range(NCF):
                        ps = psA.tile([P, 512], F32, tag="seg")
                        for b in range(NB):
                            nc.tensor.matmul(
                                ps[:],
                                z_i[b][:, d * P : (d + 1) * P],
                                oh[b][:, cf * 512 : (cf + 1) * 512],
                                start=(b == 0),
                                stop=(b == NB - 1),
                            )
                        so = work3.tile([P, 512], BF16, tag="segout", bufs=4)
                        nc.scalar.copy(so[:], ps[:])
                        dma_eng = nc.sync if (d * NCF + cf) % 2 == 0 else nc.scalar
                        dma_eng.dma_start(
                            cc_in[d * P : (d + 1) * P, cf * 512 : (cf + 1) * 512],
                            so[:],
                        )


            # ---------------- phase B (overlaps collective) ----------------
            zjt = [zjtp.tile([P, BL], BF16, name=f"zjt{d}") for d in range(ND)]
            with tc.tile_pool(name="embt", bufs=1) as embtp:
                embT = [embtp.tile([P, BL], F32, name=f"embT{d}") for d in range(ND)]
                acc = embtp.tile([P, BL], F32, name="acc")
                for d in range(ND):
                    # gate emb_jT transfers behind the last emb_i load so
                    # phase A input DMAs get the full HBM bandwidth first
                    nc.vector.tensor_copy(embT[d][0:1, 0:1], e_last[0:1, 0:1])
                    nc.gpsimd.dma_start(embT[d][:], emb_jT[d * P : (d + 1) * P, :])
                    sq2 = work.tile([P, BL], F32, tag="sqscr2")
                    nc.scalar.activation(sq2[:], embT[d][:], AF.Square)
                    if d == 0:
                        nc.vector.tensor_copy(acc[:], sq2[:])
                    else:
                        nc.vector.tensor_add(acc[:], acc[:], sq2[:])
                nrm2 = embtp.tile([P, BL], F32, name="nrm2")
                nc.gpsimd.partition_all_reduce(
                    nrm2[:], acc[:], channels=P, reduce_op=bass_isa.ReduceOp.add
                )
                nc.scalar.activation(nrm2[:], nrm2[:], AF.Sqrt)
                nc.vector.tensor_scalar(nrm2[:], nrm2[:], EPS, None, ALU.max)
                invb = embtp.tile([P, BL], F32, name="invb")
                nc.vector.reciprocal(invb[:], nrm2[:])
                for d in range(ND):
                    nc.vector.tensor_tensor(zjt[d][:], embT[d][:], invb[:], ALU.mult)

            nc.gpsimd.collective_compute(
                "AllReduce",
                ALU.add,
                replica_groups=[list(range(N_CORES))],
                ins=[cc_in[:].opt()],
                outs=[cc_out[:].opt()],
            )

            # ---------------- phase C ----------------
            with (
                tc.tile_pool(name="phC", bufs=1) as pcpool,
                tc.tile_pool(name="psC", bufs=2, space="PSUM") as psC,
                tc.tile_pool(name="psFin", bufs=1, space="PSUM") as psFin,
                tc.tile_pool(name="psSim", bufs=5, space="PSUM") as psSim,
            ):
                segT = [pcpool.tile([P, C], BF16, name=f"segT{d}") for d in range(ND)]
                sq_all = [pcpool.tile([P, C], BF16, name=f"sq{d}") for d in range(ND)]
                for d in range(ND):
                    dma_eng = (nc.sync, nc.scalar, nc.gpsimd)[d % 3]
                    dma_eng.dma_start(segT[d][:], cc_out[d * P : (d + 1) * P, :])
                    nc.vector.tensor_tensor(
                        sq_all[d][:], segT[d][:], segT[d][:], ALU.mult
                    )
                cnt_rowb = constp.tile([1, C], BF16)
                nc.sync.dma_start(cnt_rowb[:], cc_out[D : D + 1, :])
                cnt_row2 = constp.tile([1, C], F32)
                nc.vector.tensor_copy(cnt_row2[:], cnt_rowb[:])
                ident1 = constp.tile([1, 1], F32)
                nc.vector.memset(ident1[:], 1.0)
                cnt_col = constp.tile([P, NCC], F32)
                ssq_col = constp.tile([P, NCC], F32)
                ic = constp.tile([P, NCC], F32)
                scale_col = constp.tile([P, NCC], F32)
                ic2 = constp.tile([P, NCC], F32)
                bias_col = constp.tile([P, NCC], F32)

                def emit_bias_prep():
                    # PE ops (transposes + ssq matmuls) emitted AFTER the
                    # first 4 sim blocks so the sim matmul stream starts at
                    # the first segT chunk; 4 blocks < 5 psSim slots keeps
                    # this deadlock-free (block 5+ waits Sqrt[0] -> bias ->
                    # these ops, which precede it on the PE queue).
                    for cc in range(NCC):
                        pt = psC.tile([P, 1], F32, tag="col1", name=f"pt{cc}")
                        nc.tensor.transpose(
                            pt[:], cnt_row2[0:1, cc * P : (cc + 1) * P], ident1[:]
                        )
                        nc.vector.tensor_copy(cnt_col[:, cc : cc + 1], pt[:])
                    for cc in range(NCC):
                        pq = psC.tile([P, 1], F32, tag="col1", name=f"pq{cc}")
                        for d in range(ND):
                            nc.tensor.matmul(
                                pq[:],
                                sq_all[d][:, cc * P : (cc + 1) * P],
                                ones_bf[:],
                                start=(d == 0),
                                stop=(d == ND - 1),
                            )
                        nc.vector.tensor_copy(ssq_col[:, cc : cc + 1], pq[:])
                    nc.vector.reciprocal(ic[:], cnt_col[:])
                    nc.vector.tensor_scalar(
                        scale_col[:], ic[:], -2.0, None, ALU.mult
                    )
                    nc.vector.tensor_tensor(ic2[:], ic[:], ic[:], ALU.mult)
                    nc.vector.tensor_tensor(
                        bias_col[:], ssq_col[:], ic2[:], ALU.mult
                    )
                    nc.vector.tensor_scalar(
                        bias_col[:], bias_col[:], 1.0, None, ALU.add
                    )

                sp_st = constp.tile([P, NBLK], F32)
                dg_st = constp.tile([P, NBLK], F32)
                sp_dump = dump.tile([P, 512], F32, name="sp_dump")
                with tc.tile_pool(name="rall", bufs=1) as rallp:
                    r_all = [
                        rallp.tile([P, 512], F32, name=f"r{blk}")
                        for blk in range(NBLK)
                    ]
                    # pass 1: matmuls per block; the first 4 blocks' Sqrts
                    # are HELD until after emit_bias_prep so bias_col is
                    # written before any read in trace order. 4 held psum
                    # tiles < 5 psSim slots keeps the PE queue deadlock-free.
                    def emit_sqrt_diag(blk, cc, bf, ps):
                        nc.scalar.activation(
                            r_all[blk][:],
                            ps[:],
                            AF.Sqrt,
                            bias=bias_col[:, cc : cc + 1],
                            scale=scale_col[:, cc : cc + 1],
                        )
                        # diag term in one fused DVE op:
                        # (label == c) * r, accumulated along b
                        prod = work.tile(
                            [P, 512], F32, tag="prod", name=f"prod{blk}"
                        )
                        nc.vector.scalar_tensor_tensor(
                            prod[:],
                            lab_bc[:, bf * 512 : (bf + 1) * 512],
                            ccol_t[:, cc : cc + 1],
                            r_all[blk][:],
                            op0=ALU.is_equal,
                            op1=ALU.mult,
                            accum_out=dg_st[:, blk : blk + 1],
                        )

                    held = []
                    for cc in range(NCC):
                        if cc == 2:
                            emit_bias_prep()
                            for h_blk, h_cc, h_bf, h_ps in held:
                                emit_sqrt_diag(h_blk, h_cc, h_bf, h_ps)
                            held = []
                        for bf in range(NBF):
                            blk = cc * NBF + bf
                            ps = psSim.tile([P, 512], F32, tag="sim")
                            for d in range(ND):
                                nc.tensor.matmul(
                                    ps[:],
                                    segT[d][:, cc * P : (cc + 1) * P],
                                    zjt[d][:, bf * 512 : (bf + 1) * 512],
                                    start=(d == 0),
                                    stop=(d == ND - 1),
                                )
                            if cc < 2:
                                held.append((blk, cc, bf, ps))
                            else:
                                emit_sqrt_diag(blk, cc, bf, ps)
                    # pass 2: Exp + Ln batched (one table switch total).
                    # two_gate copies two_col after the last Sqrt so every Exp
                    # data-depends on all Sqrts -> scheduler cannot interleave
                    # Exp/Ln into the Sqrt stretch (would thrash ACT tables).
                    two_gate = constp.tile([P, 1], F32)
                    gate_probe = work.tile([P, 1], F32, tag="gateprobe")
                    nc.vector.tensor_reduce(
                        gate_probe[:], r_all[NBLK - 1][:, 0:2], axis=AX.X, op=ALU.max
                    )
                    nc.vector.tensor_scalar(
                        two_gate[:], gate_probe[:], 0.0, 2.0, ALU.mult, ALU.add
                    )
                    ex_all = [
                        rallp.tile([P, 512], F32, name=f"ex{blk}")
                        for blk in range(NBLK)
                    ]
                    # softplus sum via a pairwise product tree: sim in [0,2]
                    # so (1+e^sim) in [2, 8.4] and a product of 16 factors
                    # stays under 6e14 (fp32-safe). 31 full-width DVE ops
                    # (16 adds + 15 mults) run under the Exp stream, then a
                    # SINGLE Ln+accum replaces the 16-Ln batch. The fold
                    # chain itself orders Ln after all Exps (no gate needed).
                    for blk in range(NBLK):
                        nc.scalar.activation(
                            ex_all[blk][:],
                            r_all[blk][:],
                            AF.Exp,
                            bias=two_gate[:],
                            scale=-1.0,
                        )
                        nc.vector.tensor_scalar(
                            ex_all[blk][:], ex_all[blk][:], 1.0, None, ALU.add
                        )
                    step = 1
                    while step < NBLK:
                        for k in range(0, NBLK, 2 * step):
                            nc.vector.tensor_tensor(
                                ex_all[k][:],
                                ex_all[k][:],
                                ex_all[k + step][:],
                                ALU.mult,
                            )
                        step *= 2
                    nc.scalar.activation(
                        sp_dump[:],
                        ex_all[0][:],
                        AF.Ln,
                        bias=0.0,
                        accum_out=sp_st[:, 0:1],
                    )

                # diag reduction first (dg_st is complete after pass 1, so
                # this overlaps the Exp/Ln batches); sp reduction is the tail
                pf2 = psFin.tile([1, NBLK], F32, tag="fin")
                nc.tensor.matmul(pf2[:], ones_col[:], dg_st[:], start=True, stop=True)
                dg_row = constp.tile([1, NBLK], F32)
                nc.vector.tensor_copy(dg_row[:], pf2[:])
                dg_tot = constp.tile([1, 1], F32)
                nc.vector.tensor_reduce(dg_tot[:], dg_row[:], axis=AX.X, op=ALU.add)
                nc.sync.dma_start(out_partial[0:1, 1:2], dg_tot[:])

                pf = psFin.tile([1, 1], F32, tag="fin")
                nc.tensor.matmul(
                    pf[:], ones_col[:], sp_st[:, 0:1], start=True, stop=True
                )
                sp_tot = constp.tile([1, 1], F32)
                nc.vector.tensor_copy(sp_tot[:], pf[:])
                nc.sync.dma_start(out_partial[0:1, 0:1], sp_tot[:])

    nc.compile()
    _NC_CACHE["nc"] = nc
    return nc


def make_in_maps(emb_i, emb_j, labels):
    emb_i = np.ascontiguousarray(np.asarray(emb_i, dtype=np.float32))
    emb_j = np.ascontiguousarray(np.asarray(emb_j, dtype=np.float32))
    labf = np.asarray(labels).astype(np.float32)
    iota_bc = np.ascontiguousarray(
        np.broadcast_to(np.arange(C, dtype=np.float32)[None, :], (P, C))
    )
    ccol = np.ascontiguousarray(
        np.arange(P, dtype=np.float32)[:, None]
        + P * np.arange(NCC, dtype=np.float32)[None, :]
    )
    in_maps = []
    for k in range(N_CORES):
        sl = slice(k * BL, (k + 1) * BL)
        lab_k = labf[sl]
        in_maps.append(
            {
                "emb_i": emb_i[sl],
                "emb_jT": np.ascontiguousarray(emb_j[sl].T),
                "labels_colmat": np.ascontiguousarray(lab_k.reshape(NB, P).T),
                "label_bcast": np.ascontiguousarray(
                    np.broadcast_to(lab_k[None, :], (P, BL))
                ),
                "iota_bcast": iota_bc,
                "ccol": ccol,
            }
        )
    return in_maps


def combine_partials(results):
    tot = 0.0
    for k in range(N_CORES):
        p = np.asarray(results[k]["out_partial"], dtype=np.float64)
        tot += p[0, 0] + p[0, 1]
    loss = (tot - 2.0 * B) / (B * C)
    return np.asarray(np.float32(loss))


def run(emb_i, emb_j, labels, **run_kwargs):
    nc = build_nc()
    in_maps = make_in_maps(emb_i, emb_j, labels)
    res = bass_utils.run_bass_kernel_spmd(
        nc, in_maps, core_ids=list(range(N_CORES)), **run_kwargs
    )
    return combine_partials(res.results), res


def kernel(emb_i, emb_j, labels):
    loss, _ = run(emb_i, emb_j, labels)
    return loss



# revision 8
# speedup vs baseline: 1.2366x; 1.2366x over previous
"""Trainium2 Bass kernel for nn_BCELoss_64330020159675 (segment_reduce BCE loss).

Class-bucketed data-parallel layout over 8 NeuronCores:
  Host: labels are a permutation of arange(B) % C, so each 128-class window
  has exactly B/8 rows. emb_i rows are bucket-sorted so core k receives the
  rows whose label falls in window k (padded with zero rows if short) while
  emb_j keeps the natural batch slice. Per-class count-derived coefficients
  (-2/cnt, 1/cnt^2) are precomputed on host from labels alone.

  phase A (per core): normalize local emb_i rows; onehot over the LOCAL
    128-class window; segT_k[d, c_loc] = sum_b z_i[b, d] oh[b, c_loc] via
    64 N=128 matmuls (8 psum tiles, accumulated over batch chunks);
    ssq_k[c_loc] = |seg_c|^2 via squares + ones-matmuls. seg (bf16) + ssq
    ride ONE AllGather of [D+1, 128] per rank (256 KB) -> [8*(D+1), 128].
  phase B (overlaps AG): load emb_jT, squares (gpsimd/ACT split), column
    norms via ones-matmuls into psum, sqrt on ACT, reciprocal_approx_fast,
    PE broadcast row->128 partitions, z_jT = emb_jT * inv (bf16).
  phase C: Q[c,b] = sum_d segT[d,c] z_jT[d,b] (128 N=512 matmuls);
    r = sqrt(Q*(-2/cnt_c) + 1 + ssq_c/cnt_c^2) folded into the Sqrt
    activation's per-partition scale/bias; diag via one fused
    scalar_tensor_tensor per block; softplus(2 - r) with accum_out per
    block (batched after all Sqrts: one table switch).
  Host: loss = (sum_cores(sp_total + diag_total) - 2B) / (B*C).

Identity: BCEWithLogits sum = sum softplus(sim) - sum match*sim,
and sum(match*sim) = 2*B - sum_b r[b, label_b].
"""
import numpy as np
import ml_dtypes

import concourse.bacc as bacc
import concourse.mybir as mybir
import concourse.tile as tile
from concourse import bass_utils

B = 8192
D = 1024
C = 1024
N_CORES = 8
BL = B // N_CORES          # 1024 natural batch rows per core (emb_j side)
CW = C // N_CORES          # 128 classes owned per core (emb_i side)
P = 128
NB = BL // P               # 8 batch chunks
ND = D // P                # 8 d chunks
NCC = C // P               # 8 class chunks in phase C
NBF = BL // 512            # 2 batch free-dim chunks
NBLK = NCC * NBF           # 16 sim blocks
ROWS = D + 1               # AG payload rows per rank (seg d-major + ssq row)
EPS = 1e-12

F32 = mybir.dt.float32
BF16 = mybir.dt.bfloat16
AF = mybir.ActivationFunctionType
ALU = mybir.AluOpType
AX = mybir.AxisListType

_NC_CACHE = {}


def build_nc():
    if "nc" in _NC_CACHE:
        return _NC_CACHE["nc"]

    nc = bacc.Bacc(
        "TRN2", target_bir_lowering=False, debug=False, num_devices=N_CORES
    )
    emb_i = nc.dram_tensor("emb_i", [BL, D], F32, kind="ExternalInput")
    emb_jT = nc.dram_tensor("emb_jT", [D, BL], F32, kind="ExternalInput")
    loclab_cm = nc.dram_tensor("loclab_cm", [P, NB], F32, kind="ExternalInput")
    iota128 = nc.dram_tensor("iota128", [P, P], F32, kind="ExternalInput")
    label_bcast = nc.dram_tensor("label_bcast", [P, BL], F32, kind="ExternalInput")
    ccol = nc.dram_tensor("ccol", [P, NCC], F32, kind="ExternalInput")
    sc_col = nc.dram_tensor("sc_col", [P, NCC], F32, kind="ExternalInput")
    ic2_col = nc.dram_tensor("ic2_col", [P, NCC], F32, kind="ExternalInput")
    eye8 = nc.dram_tensor("eye8", [8, 8], BF16, kind="ExternalInput")
    out_partial = nc.dram_tensor("out_partial", [1, 2], F32, kind="ExternalOutput")

    with tile.TileContext(nc) as tc:
        with (
            tc.tile_pool(name="dram", bufs=1, space="DRAM") as dram,
            tc.tile_pool(name="const", bufs=1) as constp,
            tc.tile_pool(name="zjt", bufs=1) as zjtp,
            tc.tile_pool(name="work", bufs=2) as work,
            tc.tile_pool(name="dump", bufs=1) as dump,
        ):
            cc_in = dram.tile([ROWS, CW], BF16)
            cc_ag = dram.tile([N_CORES * ROWS, CW], BF16, addr_space="Shared")

            ones_bf = constp.tile([P, 1], BF16)
            nc.vector.memset(ones_bf[:], 1.0)
            ones_col = constp.tile([P, 1], F32)
            nc.vector.memset(ones_col[:], 1.0)
            ones_row = constp.tile([1, P], F32)
            nc.vector.memset(ones_row[:], 1.0)

            lab_cm = constp.tile([P, NB], F32)
            nc.gpsimd.dma_start(lab_cm[:], loclab_cm[:])
            iota_t = constp.tile([P, P], F32)
            nc.gpsimd.dma_start(iota_t[:], iota128[:])
            lab_bc = constp.tile([P, BL], F32)
            nc.scalar.dma_start(lab_bc[:], label_bcast[:])
            ccol_t = constp.tile([P, NCC], F32)
            nc.scalar.dma_start(ccol_t[:], ccol[:])
            sc_t = constp.tile([P, NCC], F32)
            nc.scalar.dma_start(sc_t[:], sc_col[:])
            ic2_t = constp.tile([P, NCC], F32)
            nc.scalar.dma_start(ic2_t[:], ic2_col[:])
            eye8_t = constp.tile([8, 8], BF16)
            nc.scalar.dma_start(eye8_t[:], eye8[:])

            # ---------------- phase A ----------------
            sq_dump = dump.tile([P, D], BF16, name="sq_dump")
            with (
                tc.tile_pool(name="phA", bufs=1) as pa,
                tc.tile_pool(name="psA", bufs=1, space="PSUM") as psA,
            ):
                z_i = [pa.tile([P, D], BF16, name=f"zi{b}") for b in range(NB)]
                oh = [pa.tile([P, P], BF16, name=f"oh{b}") for b in range(NB)]
                psa_bank = [psA.tile([P, 512], F32, name=f"psab{i}") for i in range(2)]
                psa = [
                    psa_bank[d // 4][:, (d % 4) * P : (d % 4 + 1) * P]
                    for d in range(ND)
                ]
                for b in range(NB):
                    e = work.tile([P, D], F32, tag="embi", bufs=4)
                    dma_eng = (nc.sync, nc.scalar, nc.gpsimd)[b % 3]
                    dma_eng.dma_start(e[:], emb_i[b * P : (b + 1) * P, :])
                    ss = work.tile([P, 1], F32, tag="ss")
                    nc.scalar.activation(sq_dump[:], e[:], AF.Square, accum_out=ss[:])
                    nrm = work.tile([P, 1], F32, tag="nrm")
                    nc.scalar.activation(nrm[:], ss[:], AF.Sqrt)
                    nc.vector.tensor_scalar(nrm[:], nrm[:], EPS, None, ALU.max)
                    inv = work.tile([P, 1], F32, tag="inv")
                    nc.vector.reciprocal(inv[:], nrm[:])
                    nc.vector.tensor_scalar(z_i[b][:], e[:], inv[:], None, ALU.mult)
                    nc.vector.tensor_scalar(
                        oh[b][:], iota_t[:], lab_cm[:, b : b + 1], None, ALU.is_equal
                    )
                    for d in range(ND):
                        # 4 dd-slices share one psum bank = one zero region,
                        # so the whole bank is ONE accumulation group: start
                        # zeroes the full 2KB on the first matmul into the
                        # bank, stop closes it on the last.
                        nc.tensor.matmul(
                            psa[d],
                            z_i[b][:, d * P : (d + 1) * P],
                            oh[b][:],
                            start=(b == 0 and d % 4 == 0),
                            stop=(b == NB - 1 and d % 4 == 3),
                        )

                # seg (bf16) out + local ssq row, both into the AG payload
                ssq_ps = psA.tile([1, CW], F32, name="ssq_ps")
                for d in range(ND):
                    seg_sb = work.tile([P, P], BF16, tag="segsb", bufs=4)
                    nc.vector.tensor_copy(seg_sb[:], psa[d])
                    sq_dd = work.tile([P, P], BF16, tag="sqdd", bufs=2)
                    nc.vector.tensor_tensor(sq_dd[:], seg_sb[:], seg_sb[:], ALU.mult)
                    nc.tensor.matmul(
                        ssq_ps[:],
                        ones_bf[:],
                        sq_dd[:],
                        start=(d == 0),
                        stop=(d == ND - 1),
                    )
                    dma_eng = nc.sync if d % 2 == 0 else nc.scalar
                    dma_eng.dma_start(cc_in[d * P : (d + 1) * P, :], seg_sb[:])
                ssq_sb = work.tile([1, CW], BF16, tag="ssqsb")
                nc.vector.tensor_copy(ssq_sb[:], ssq_ps[:])
                nc.gpsimd.dma_start(cc_in[D : D + 1, :], ssq_sb[:])

            nc.gpsimd.collective_compute(
                "AllGather",
                ALU.bypass,
                replica_groups=[list(range(N_CORES))],
                ins=[cc_in[:].opt()],
                outs=[cc_ag[:].opt()],
            )

            # ---------------- phase B (overlaps collective) ----------------
            zjt = [zjtp.tile([P, BL], BF16, name=f"zjt{d}") for d in range(ND)]
            with (
                tc.tile_pool(name="embt", bufs=1) as embtp,
                tc.tile_pool(name="psB", bufs=1, space="PSUM") as psB,
            ):
                embT = [embtp.tile([P, BL], F32, name=f"embT{d}") for d in range(ND)]
                sqj = [embtp.tile([P, BL], BF16, name=f"sqj{d}") for d in range(ND)]
                nj_ps = [psB.tile([1, 512], F32, name=f"nj{h}") for h in range(NBF)]
                for d in range(ND):
                    dma_eng = (nc.sync, nc.scalar, nc.gpsimd)[d % 3]
                    dma_eng.dma_start(embT[d][:], emb_jT[d * P : (d + 1) * P, :])
                    sq_eng = nc.gpsimd if d % 2 == 0 else nc.scalar
                    if d % 2 == 0:
                        nc.gpsimd.tensor_tensor(
                            sqj[d][:], embT[d][:], embT[d][:], ALU.mult
                        )
                    else:
                        nc.scalar.activation(sqj[d][:], embT[d][:], AF.Square)
                    for h in range(NBF):
                        nc.tensor.matmul(
                            nj_ps[h][:],
                            ones_bf[:],
                            sqj[d][:, h * 512 : (h + 1) * 512],
                            start=(d == 0),
                            stop=(d == ND - 1),
                        )
                nrm_row = embtp.tile([1, BL], F32, name="nrm_row")
                for h in range(NBF):
                    nc.scalar.activation(
                        nrm_row[:, h * 512 : (h + 1) * 512], nj_ps[h][:], AF.Sqrt
                    )
                inv_row = embtp.tile([1, BL], F32, name="inv_row")
                nc.vector.reciprocal_approx_fast(inv_row[:], nrm_row[:])
                bc_ps = [psB.tile([P, 512], F32, name=f"bc{h}") for h in range(NBF)]
                for h in range(NBF):
                    nc.tensor.matmul(
                        bc_ps[h][:],
                        ones_row[:],
                        inv_row[:, h * 512 : (h + 1) * 512],
                        start=True,
                        stop=True,
                    )
                for d in range(ND):
                    for h in range(NBF):
                        nc.vector.tensor_tensor(
                            zjt[d][:, h * 512 : (h + 1) * 512],
                            embT[d][:, h * 512 : (h + 1) * 512],
                            bc_ps[h][:],
                            ALU.mult,
                        )

            # ---------------- phase C ----------------
            with (
                tc.tile_pool(name="phC", bufs=1) as pcpool,
                tc.tile_pool(name="psC", bufs=1, space="PSUM") as psC,
                tc.tile_pool(name="psSim", bufs=5, space="PSUM") as psSim,
            ):
                # segT[d][p, k*CW + c] <- cc_ag[k*ROWS + d*P + p, c]
                segT = [pcpool.tile([P, C], BF16, name=f"segT{d}") for d in range(ND)]
                ag_view = cc_ag[:, :].rearrange("(k r) c -> r k c", k=N_CORES)
                for d in range(ND):
                    dma_eng = (nc.sync, nc.scalar, nc.gpsimd)[d % 3]
                    dma_eng.dma_start(
                        segT[d][:, :].rearrange("p (k c) -> p k c", k=N_CORES),
                        ag_view[d * P : (d + 1) * P, :, :],
                    )
                # ssq rows of all ranks -> [8, CW] -> transpose -> [P, NCC]
                ssqr8 = pcpool.tile([8, CW], BF16, name="ssqr8")
                ag_rows = cc_ag[:, :].rearrange("(k r) c -> k r c", k=N_CORES)
                nc.gpsimd.dma_start(ssqr8[:], ag_rows[:, D : D + 1, :].opt())
                tp_ps = psC.tile([P, NCC], BF16, name="tp_ps")
                nc.tensor.transpose(tp_ps[:], ssqr8[:], eye8_t[:])
                ssq_col = constp.tile([P, NCC], F32)
                nc.vector.tensor_copy(ssq_col[:], tp_ps[:])
                bias_col = constp.tile([P, NCC], F32)
                nc.vector.tensor_tensor(bias_col[:], ssq_col[:], ic2_t[:], ALU.mult)
                nc.vector.tensor_scalar(bias_col[:], bias_col[:], 1.0, None, ALU.add)

                sp_st = constp.tile([P, NBLK], F32)
                dg_st = constp.tile([P, NBLK], F32)
                sp_dump = dump.tile([P, 512], BF16, name="sp_dump")
                r_all = [
                    pcpool.tile([P, 512], BF16, name=f"r{blk}") for blk in range(NBLK)
                ]
                for cc in range(NCC):
                    for bf in range(NBF):
                        blk = cc * NBF + bf
                        ps = psSim.tile([P, 512], F32, tag="sim")
                        for d in range(ND):
                            nc.tensor.matmul(
                                ps[:],
                                segT[d][:, cc * P : (cc + 1) * P],
                                zjt[d][:, bf * 512 : (bf + 1) * 512],
                                start=(d == 0),
                                stop=(d == ND - 1),
                            )
                        nc.scalar.activation(
                            r_all[blk][:],
                            ps[:],
                            AF.Sqrt,
                            bias=bias_col[:, cc : cc + 1],
                            scale=sc_t[:, cc : cc + 1],
                        )
                        prod = work.tile([P, 512], BF16, tag="prod", bufs=2)
                        nc.vector.scalar_tensor_tensor(
                            prod[:],
                            lab_bc[:, bf * 512 : (bf + 1) * 512],
                            ccol_t[:, cc : cc + 1],
                            r_all[blk][:],
                            op0=ALU.is_equal,
                            op1=ALU.mult,
                            accum_out=dg_st[:, blk : blk + 1],
                        )
                # pass 2: Exp batched behind all Sqrts (gate makes every Exp
                # depend on the last Sqrt -> no ACT table thrash), then the
                # softplus sum via a pairwise product tree: sim in [0,2] so
                # (1+e^sim) in [2,8.4]; a product of 16 factors stays under
                # 6e14 (fp32-safe) and a SINGLE Ln+accum replaces 16 Lns.
                gate2 = constp.tile([P, 1], F32)
                nc.vector.tensor_scalar(
                    gate2[:], r_all[NBLK - 1][:, 0:1], 0.0, 2.0, ALU.mult, ALU.add
                )
                ex_all = [
                    pcpool.tile([P, 512], F32, name=f"ex{blk}")
                    for blk in range(NBLK)
                ]
                for blk in range(NBLK):
                    nc.scalar.activation(
                        ex_all[blk][:],
                        r_all[blk][:],
                        AF.Exp,
                        bias=gate2[:],
                        scale=-1.0,
                    )
                    nc.vector.tensor_scalar(
                        ex_all[blk][:], ex_all[blk][:], 1.0, None, ALU.add
                    )
                step = 1
                while step < NBLK:
                    for kk in range(0, NBLK, 2 * step):
                        # first tree level on gpsimd (otherwise idle here)
                        eng = nc.gpsimd if step == 1 else nc.vector
                        eng.tensor_tensor(
                            ex_all[kk][:],
                            ex_all[kk][:],
                            ex_all[kk + step][:],
                            ALU.mult,
                        )
                    step *= 2
                nc.scalar.activation(
                    sp_dump[:],
                    ex_all[0][:],
                    AF.Ln,
                    bias=0.0,
                    accum_out=sp_st[:, 0:1],
                )

                # final reductions: diag first (complete after pass 1)
                psFin = psC
                pf2 = psFin.tile([1, NBLK], F32, name="fin_dg")
                nc.tensor.matmul(pf2[:], ones_col[:], dg_st[:], start=True, stop=True)
                dg_row = constp.tile([1, NBLK], F32)
                nc.vector.tensor_copy(dg_row[:], pf2[:])
                dg_tot = constp.tile([1, 1], F32)
                nc.vector.tensor_reduce(dg_tot[:], dg_row[:], axis=AX.X, op=ALU.add)
                nc.sync.dma_start(out_partial[0:1, 1:2], dg_tot[:])

                pf = psFin.tile([1, NBLK], F32, name="fin_sp")
                nc.tensor.matmul(
                    pf[0:1, 0:1], ones_col[:], sp_st[:, 0:1], start=True, stop=True
                )
                sp_tot = constp.tile([1, 1], F32)
                nc.vector.tensor_copy(sp_tot[:], pf[0:1, 0:1])
                nc.sync.dma_start(out_partial[0:1, 0:1], sp_tot[:])

    nc.compile()
    _NC_CACHE["nc"] = nc
    return nc


def make_in_maps(emb_i, emb_j, labels):
    emb_i = np.ascontiguousarray(np.asarray(emb_i, dtype=np.float32))
    emb_j = np.ascontiguousarray(np.asarray(emb_j, dtype=np.float32))
    lab = np.asarray(labels).astype(np.int64)

    cnt = np.bincount(lab, minlength=C).astype(np.float64)
    sc = (-2.0 / cnt).astype(np.float32)
    ic2 = (1.0 / (cnt * cnt)).astype(np.float32)

    def colmat(v):
        # [C] -> [P, NCC]: value for class cc*P + p lands at [p, cc]
        return np.ascontiguousarray(v.reshape(NCC, P).T)

    iota = np.ascontiguousarray(
        np.broadcast_to(np.arange(P, dtype=np.float32)[None, :], (P, P))
    )
    ccol = np.ascontiguousarray(
        np.arange(P, dtype=np.float32)[:, None]
        + P * np.arange(NCC, dtype=np.float32)[None, :]
    )
    sc_cm = colmat(sc)
    ic2_cm = colmat(ic2)
    eye8 = np.eye(8, dtype=ml_dtypes.bfloat16)

    in_maps = []
    for k in range(N_CORES):
        sel = np.nonzero((lab >= k * CW) & (lab < (k + 1) * CW))[0]
        assert len(sel) <= BL, f"bucket {k} overflow: {len(sel)}"
        ei = np.zeros((BL, D), dtype=np.float32)
        ei[: len(sel)] = emb_i[sel]
        ll = np.zeros((BL,), dtype=np.float32)
        ll[: len(sel)] = (lab[sel] - k * CW).astype(np.float32)

        sl = slice(k * BL, (k + 1) * BL)
        lab_k = lab[sl].astype(np.float32)
        in_maps.append(
            {
                "emb_i": ei,
                "emb_jT": np.ascontiguousarray(emb_j[sl].T),
                "loclab_cm": np.ascontiguousarray(ll.reshape(NB, P).T),
                "iota128": iota,
                "label_bcast": np.ascontiguousarray(
                    np.broadcast_to(lab_k[None, :], (P, BL))
                ),
                "ccol": ccol,
                "sc_col": sc_cm,
                "ic2_col": ic2_cm,
                "eye8": eye8,
            }
        )
    return in_maps


def combine_partials(results):
    tot = 0.0
    for k in range(N_CORES):
        p = np.asarray(results[k]["out_partial"], dtype=np.float64)
        tot += p[0, 0] + p[0, 1]
    loss = (tot - 2.0 * B) / (B * C)
    return np.asarray(np.float32(loss))


def _numpy_fallback(emb_i, emb_j, labels):
    emb_i = np.asarray(emb_i, dtype=np.float64)
    emb_j = np.asarray(emb_j, dtype=np.float64)
    lab = np.asarray(labels).astype(np.int64)
    zi = emb_i / np.maximum(np.linalg.norm(emb_i, axis=1, keepdims=True), EPS)
    zj = emb_j / np.maximum(np.linalg.norm(emb_j, axis=1, keepdims=True), EPS)
    cnt = np.bincount(lab, minlength=C).astype(np.float64)
    seg = np.zeros((C, D))
    np.add.at(seg, lab, zi)
    proto = seg / cnt[:, None]
    d2 = (
        (zj * zj).sum(1)[:, None]
        + (proto * proto).sum(1)[None, :]
        - 2.0 * zj @ proto.T
    )
    sim = 2.0 - np.sqrt(np.maximum(d2, 0.0))
    match = (np.arange(C)[None, :] == lab[:, None]).astype(np.float64)
    sp = np.logaddexp(0.0, sim)
    loss = np.mean(sp - match * sim)
    return np.asarray(np.float32(loss))


def run(emb_i, emb_j, labels, **run_kwargs):
    nc = build_nc()
    in_maps = make_in_maps(emb_i, emb_j, labels)
    res = bass_utils.run_bass_kernel_spmd(
        nc, in_maps, core_ids=list(range(N_CORES)), **run_kwargs
    )
    return combine_partials(res.results), res


def kernel(emb_i, emb_j, labels):
    lab = np.asarray(labels).astype(np.int64)
    sizes = np.bincount(lab // CW, minlength=N_CORES)
    if sizes.max() > BL or np.bincount(lab, minlength=C).min() == 0:
        return _numpy_fallback(emb_i, emb_j, labels)
    loss, _ = run(emb_i, emb_j, labels)
    return loss


# revision 19
# speedup vs baseline: 1.4072x; 1.1380x over previous
"""Trainium2 Bass kernel for nn_BCELoss_64330020159675 (segment_reduce BCE loss).

Class-bucketed data-parallel layout over 8 NeuronCores:
  Host: labels are a permutation of arange(B) % C, so each 128-class window
  has exactly B/8 rows. emb_i rows are bucket-sorted so core k receives the
  rows whose label falls in window k (padded with zero rows if short) while
  emb_j keeps the natural batch slice. Per-class count-derived coefficients
  (-2/cnt, 1/cnt^2) are precomputed on host from labels alone.

  phase A (per core): normalize local emb_i rows; onehot over the LOCAL
    128-class window; segT_k[d, c_loc] = sum_b z_i[b, d] oh[b, c_loc] via
    64 N=128 matmuls (2 psum banks, one accumulation group per bank);
    ssq_k[c_loc] = |seg_c|^2 via squares + ones-matmuls. seg (fp8e4) +
    ssq (bf16, bitcast into 2 fp8 rows) ride ONE AllGather of [D+2, 128]
    fp8 per rank (131 KB) -> [8*(D+2), 128].
  phase B (overlaps AG): load emb_jT, squares (gpsimd/ACT split), column
    norms via ones-matmuls into psum, sqrt on ACT, reciprocal_approx_fast,
    PE broadcast row->128 partitions, z_jT = emb_jT * inv (fp8e4).
  phase C: Q[c,b] = sum_d segT[d,c] z_jT[d,b] (128 fp8 N=512 matmuls);
    r = sqrt(Q*(-2/cnt_c) + 1 + ssq_c/cnt_c^2) folded into the Sqrt
    activation's per-partition scale/bias; diag via one fused
    scalar_tensor_tensor per block. Softplus sum via exp + a RUNNING
    product acc = (e^sim + 1) * acc (one fused STT per block, fp32-safe:
    16 factors in [2, 8.4] stay under 6e14) and a single Ln + accum.
    sqrt/exp alternate in two 8-block halves so half the exp stream hides
    under the matmul window (2 extra ACT table loads, gated to stop the
    scheduler interleaving table sets).
  Host: loss = (sum_cores(sp_total + diag_total) - 2B) / (B*C).

Identity: BCEWithLogits sum = sum softplus(sim) - sum match*sim,
and sum(match*sim) = 2*B - sum_b r[b, label_b].
"""
import numpy as np
import ml_dtypes

import concourse.bacc as bacc
import concourse.mybir as mybir
import concourse.tile as tile
from concourse import bass_utils

B = 8192
D = 1024
C = 1024
N_CORES = 8
BL = B // N_CORES          # 1024 natural batch rows per core (emb_j side)
CW = C // N_CORES          # 128 classes owned per core (emb_i side)
P = 128
NB = BL // P               # 8 batch chunks
ND = D // P                # 8 d chunks
NCC = C // P               # 8 class chunks in phase C
NBF = BL // 512            # 2 batch free-dim chunks
NBLK = NCC * NBF           # 16 sim blocks
ROWS = D + 2               # AG payload rows per rank (fp8 seg + 2 ssq rows)
EPS = 1e-12
NAUX = 160                 # merged aux input cols: iota128|loclab|ccol|sc|ic2

F32 = mybir.dt.float32
BF16 = mybir.dt.bfloat16
FP8 = mybir.dt.float8e4
AF = mybir.ActivationFunctionType
ALU = mybir.AluOpType
AX = mybir.AxisListType

_NC_CACHE = {}


def build_nc():
    if "nc" in _NC_CACHE:
        return _NC_CACHE["nc"]

    nc = bacc.Bacc(
        "TRN2", target_bir_lowering=False, debug=False, num_devices=N_CORES
    )
    emb_i = nc.dram_tensor("emb_i", [BL, D], F32, kind="ExternalInput")
    emb_jT = nc.dram_tensor("emb_jT", [D, BL], F32, kind="ExternalInput")
    label_bcast = nc.dram_tensor("label_bcast", [P, BL], F32, kind="ExternalInput")
    aux = nc.dram_tensor("aux", [P, NAUX], F32, kind="ExternalInput")
    out_partial = nc.dram_tensor("out_partial", [1, 2], F32, kind="ExternalOutput")

    with tile.TileContext(nc) as tc:
        with (
            tc.tile_pool(name="dram", bufs=1, space="DRAM") as dram,
            tc.tile_pool(name="const", bufs=1) as constp,
            tc.tile_pool(name="zjt", bufs=1) as zjtp,
            tc.tile_pool(name="work", bufs=2) as work,
            tc.tile_pool(name="dump", bufs=1) as dump,
        ):
            cc_in = dram.tile([ROWS, CW], FP8)
            cc_ag = dram.tile([N_CORES * ROWS, CW], FP8, addr_space="Shared")

            ones_bf = constp.tile([P, 1], BF16)
            nc.vector.memset(ones_bf[:], 1.0)
            ones_col = constp.tile([P, 1], F32)
            nc.vector.memset(ones_col[:], 1.0)
            ones_row = constp.tile([1, P], F32)
            nc.vector.memset(ones_row[:], 1.0)
            eye8_t = constp.tile([8, 8], BF16)
            eye_ir = constp.tile([8, 8], F32)
            nc.gpsimd.iota(
                eye_ir[:],
                pattern=[[1, 8]],
                base=0,
                channel_multiplier=0,
                allow_small_or_imprecise_dtypes=True,
            )
            eye_ic = constp.tile([8, 1], F32)
            nc.gpsimd.iota(
                eye_ic[:],
                pattern=[[1, 1]],
                base=0,
                channel_multiplier=1,
                allow_small_or_imprecise_dtypes=True,
            )
            nc.vector.tensor_scalar(
                eye8_t[:], eye_ir[:], eye_ic[:, 0:1], None, ALU.is_equal
            )

            aux_t = constp.tile([P, NAUX], F32)
            nc.sync.dma_start(aux_t[:], aux[:])
            iota_t = aux_t[:, 0:128]
            lab_cm = aux_t[:, 128:136]
            ccol_t = aux_t[:, 136:144]
            sc_t = aux_t[:, 144:152]
            ic2_t = aux_t[:, 152:160]
            lab_bc = constp.tile([P, BL], F32)
            nc.scalar.dma_start(lab_bc[:], label_bcast[:])

            # ---------------- phase A ----------------
            sq_dump = dump.tile([P, D], BF16, name="sq_dump")
            with (
                tc.tile_pool(name="phA", bufs=1) as pa,
                tc.tile_pool(name="psA", bufs=1, space="PSUM") as psA,
            ):
                z_i = [pa.tile([P, D], BF16, name=f"zi{b}") for b in range(NB)]
                oh = [pa.tile([P, P], BF16, name=f"oh{b}") for b in range(NB)]
                psa_bank = [psA.tile([P, 512], F32, name=f"psab{i}") for i in range(2)]
                psa = [
                    psa_bank[d // 4][:, (d % 4) * P : (d % 4 + 1) * P]
                    for d in range(ND)
                ]
                for b in range(NB):
                    e = work.tile([P, D], F32, tag="embi", bufs=4)
                    dma_eng = (nc.sync, nc.scalar, nc.gpsimd)[b % 3]
                    dma_eng.dma_start(e[:], emb_i[b * P : (b + 1) * P, :])
                    ss = work.tile([P, 1], F32, tag="ss")
                    nc.scalar.activation(sq_dump[:], e[:], AF.Square, accum_out=ss[:])
                    nrm = work.tile([P, 1], F32, tag="nrm")
                    nc.scalar.activation(nrm[:], ss[:], AF.Sqrt)
                    nc.vector.tensor_scalar(nrm[:], nrm[:], EPS, None, ALU.max)
                    inv = work.tile([P, 1], F32, tag="inv")
                    nc.vector.reciprocal(inv[:], nrm[:])
                    nc.vector.tensor_scalar(z_i[b][:], e[:], inv[:], None, ALU.mult)
                    nc.vector.tensor_scalar(
                        oh[b][:], iota_t, lab_cm[:, b : b + 1], None, ALU.is_equal
                    )
                    for d in range(ND):
                        # 4 dd-slices share one psum bank = one zero region,
                        # so the whole bank is ONE accumulation group.
                        nc.tensor.matmul(
                            psa[d],
                            z_i[b][:, d * P : (d + 1) * P],
                            oh[b][:],
                            start=(b == 0 and d % 4 == 0),
                            stop=(b == NB - 1 and d % 4 == 3),
                        )

                # seg (fp8) out + local ssq row (bf16, bitcast to 2 fp8 rows)
                ssq_ps = psA.tile([1, CW], F32, name="ssq_ps")
                for d in range(ND):
                    seg_sb = work.tile([P, P], FP8, tag="segsb", bufs=4)
                    nc.vector.tensor_copy(seg_sb[:], psa[d])
                    sq_dd = work.tile([P, P], BF16, tag="sqdd", bufs=2)
                    nc.vector.tensor_tensor(sq_dd[:], seg_sb[:], seg_sb[:], ALU.mult)
                    nc.tensor.matmul(
                        ssq_ps[:],
                        ones_bf[:],
                        sq_dd[:],
                        start=(d == 0),
                        stop=(d == ND - 1),
                    )
                    dma_eng = nc.sync if d % 2 == 0 else nc.scalar
                    dma_eng.dma_start(cc_in[d * P : (d + 1) * P, :], seg_sb[:])
                # ssq rides the fp8 payload as a compensated hi+lo pair:
                # hi = fp8(ssq), lo = fp8(ssq - hi) -> ~7e-6 relative error
                ssq_hi = work.tile([1, CW], FP8, tag="ssqhi")
                nc.vector.tensor_copy(ssq_hi[:], ssq_ps[:])
                ssq_lo = work.tile([1, CW], FP8, tag="ssqlo")
                nc.vector.tensor_tensor(
                    ssq_lo[:], ssq_ps[:], ssq_hi[:], ALU.subtract
                )
                nc.sync.dma_start(cc_in[D : D + 1, :], ssq_hi[:])
                nc.scalar.dma_start(cc_in[D + 1 : D + 2, :], ssq_lo[:])

            nc.gpsimd.collective_compute(
                "AllGather",
                ALU.bypass,
                replica_groups=[list(range(N_CORES))],
                ins=[cc_in[:].opt()],
                outs=[cc_ag[:].opt()],
            )

            # ---------------- phase B (overlaps collective) ----------------
            zjt = [zjtp.tile([P, BL], FP8, name=f"zjt{d}") for d in range(ND)]
            with (
                tc.tile_pool(name="embt", bufs=1) as embtp,
                tc.tile_pool(name="psB", bufs=1, space="PSUM") as psB,
            ):
                embT = [embtp.tile([P, BL], F32, name=f"embT{d}") for d in range(ND)]
                sqj = [embtp.tile([P, BL], BF16, name=f"sqj{d}") for d in range(ND)]
                nj_ps = [psB.tile([1, 512], F32, name=f"nj{h}") for h in range(NBF)]
                for d in range(ND):
                    dma_eng = (nc.sync, nc.scalar, nc.gpsimd)[d % 3]
                    dma_eng.dma_start(embT[d][:], emb_jT[d * P : (d + 1) * P, :])
                    if d % 2 == 0:
                        nc.gpsimd.tensor_tensor(
                            sqj[d][:], embT[d][:], embT[d][:], ALU.mult
                        )
                    else:
                        nc.scalar.activation(sqj[d][:], embT[d][:], AF.Square)
                    for h in range(NBF):
                        nc.tensor.matmul(
                            nj_ps[h][:],
                            ones_bf[:],
                            sqj[d][:, h * 512 : (h + 1) * 512],
                            start=(d == 0),
                            stop=(d == ND - 1),
                        )
                nrm_row = embtp.tile([1, BL], F32, name="nrm_row")
                for h in range(NBF):
                    nc.scalar.activation(
                        nrm_row[:, h * 512 : (h + 1) * 512], nj_ps[h][:], AF.Sqrt
                    )
                inv_row = embtp.tile([1, BL], F32, name="inv_row")
                nc.vector.reciprocal_approx_fast(inv_row[:], nrm_row[:])
                bc_ps = [psB.tile([P, 512], F32, name=f"bc{h}") for h in range(NBF)]
                for h in range(NBF):
                    nc.tensor.matmul(
                        bc_ps[h][:],
                        ones_row[:],
                        inv_row[:, h * 512 : (h + 1) * 512],
                        start=True,
                        stop=True,
                    )
                for d in range(ND):
                    for h in range(NBF):
                        nc.vector.tensor_tensor(
                            zjt[d][:, h * 512 : (h + 1) * 512],
                            embT[d][:, h * 512 : (h + 1) * 512],
                            bc_ps[h][:],
                            ALU.mult,
                        )

            # ---------------- phase C ----------------
            with (
                tc.tile_pool(name="phC", bufs=1) as pcpool,
                tc.tile_pool(name="psC", bufs=2, space="PSUM") as psC,
                tc.tile_pool(name="psSim", bufs=5, space="PSUM") as psSim,
            ):
                # segT[d][p, k*CW + c] <- cc_ag[k*ROWS + d*P + p, c]
                # HWDGE queues only: SWDGE descriptor generation for the
                # 1024-segment pattern is too slow on the critical path.
                segT = [pcpool.tile([P, C], FP8, name=f"segT{d}") for d in range(ND)]
                ag_view = cc_ag[:, :].rearrange("(k r) c -> r k c", k=N_CORES)
                for d in range(ND):
                    dma_eng = nc.sync if d % 2 == 0 else nc.scalar
                    dma_eng.dma_start(
                        segT[d][:, :].rearrange("p (k c) -> p k c", k=N_CORES),
                        ag_view[d * P : (d + 1) * P, :, :],
                    )
                # ssq hi/lo rows of all ranks -> [8, CW] fp8 -> transpose
                ag_rows = cc_ag[:, :].rearrange("(k r) c -> k r c", k=N_CORES)
                ssqr_hi = pcpool.tile([8, CW], FP8, name="ssqr_hi")
                nc.sync.dma_start(ssqr_hi[:], ag_rows[:, D : D + 1, :].opt())
                ssqr_lo = pcpool.tile([8, CW], FP8, name="ssqr_lo")
                nc.scalar.dma_start(ssqr_lo[:], ag_rows[:, D + 1 : D + 2, :].opt())
                ssqr_sum = pcpool.tile([8, CW], BF16, name="ssqr_sum")
                nc.vector.tensor_tensor(
                    ssqr_sum[:], ssqr_hi[:], ssqr_lo[:], ALU.add
                )
                tp_ps = psC.tile([P, NCC], BF16, tag="misc", name="tp_ps")
                nc.tensor.transpose(tp_ps[:], ssqr_sum[:], eye8_t[:])
                ssq_col = constp.tile([P, NCC], F32)
                nc.vector.tensor_copy(ssq_col[:], tp_ps[:])
                bias_a = constp.tile([P, NCC], F32)
                nc.vector.tensor_tensor(bias_a[:], ssq_col[:], ic2_t, ALU.mult)
                nc.vector.tensor_scalar(bias_a[:], bias_a[:], 1.0, None, ALU.add)

                sp_st = constp.tile([P, 1], F32)
                dg_st = constp.tile([P, NBLK], F32)
                sp_dump = dump.tile([P, 512], BF16, name="sp_dump")
                acc = [pcpool.tile([P, 512], F32, name=f"acc{i}") for i in range(2)]
                nc.vector.memset(acc[0][:], 1.0)
                r_all = [
                    pcpool.tile([P, 512], BF16, name=f"r{blk}") for blk in range(NBLK)
                ]

                HALF = NBLK // 2
                bias_b = constp.tile([P, NCC], F32)
                nfold = 0
                for half in range(2):
                    bias_t = bias_a if half == 0 else bias_b
                    for blk in range(half * HALF, (half + 1) * HALF):
                        cc, bf = blk // NBF, blk % NBF
                        ps = psSim.tile([P, 512], F32, tag="sim")
                        for d in range(ND):
                            nc.tensor.matmul(
                                ps[:],
                                segT[d][:, cc * P : (cc + 1) * P],
                                zjt[d][:, bf * 512 : (bf + 1) * 512],
                                start=(d == 0),
                                stop=(d == ND - 1),
                            )
                        nc.scalar.activation(
                            r_all[blk][:],
                            ps[:],
                            AF.Sqrt,
                            bias=bias_t[:, cc : cc + 1],
                            scale=sc_t[:, cc : cc + 1],
                        )
                        prod = work.tile([P, 512], BF16, tag="prod", bufs=2)
                        nc.vector.scalar_tensor_tensor(
                            prod[:],
                            lab_bc[:, bf * 512 : (bf + 1) * 512],
                            ccol_t[:, cc : cc + 1],
                            r_all[blk][:],
                            op0=ALU.is_equal,
                            op1=ALU.mult,
                            accum_out=dg_st[:, blk : blk + 1],
                        )
                    # gate: exp bias depends on this half's LAST sqrt so the
                    # scheduler can't pull exp (different ACT table set) into
                    # the sqrt stretch.
                    gate = constp.tile([P, 1], F32)
                    nc.vector.tensor_scalar(
                        gate[:],
                        r_all[(half + 1) * HALF - 1][:, 0:1],
                        0.0,
                        2.0,
                        ALU.mult,
                        ALU.add,
                    )
                    for blk in range(half * HALF, (half + 1) * HALF):
                        ex = work.tile([P, 512], F32, tag="ex", bufs=3)
                        nc.scalar.activation(
                            ex[:], r_all[blk][:], AF.Exp, bias=gate[:], scale=-1.0
                        )
                        # running product: acc = (e^sim + 1) * acc
                        nc.vector.scalar_tensor_tensor(
                            acc[(nfold + 1) % 2][:],
                            ex[:],
                            1.0,
                            acc[nfold % 2][:],
                            op0=ALU.add,
                            op1=ALU.mult,
                        )
                        nfold += 1
                        if half == 0 and blk == HALF - 1:
                            # gate the second half's sqrts behind this half's
                            # last exp via their bias tile (ex slice * 0 + bias)
                            nc.vector.scalar_tensor_tensor(
                                bias_b[:],
                                ex[:, 0:NCC],
                                0.0,
                                bias_a[:],
                                op0=ALU.mult,
                                op1=ALU.add,
                            )
                nc.scalar.activation(
                    sp_dump[:],
                    acc[nfold % 2][:],
                    AF.Ln,
                    bias=0.0,
                    accum_out=sp_st[:, 0:1],
                )

                # final reductions: diag first (complete after the sqrt pass)
                pf2 = psC.tile([1, NBLK], F32, tag="misc", name="fin_dg")
                nc.tensor.matmul(pf2[:], ones_col[:], dg_st[:], start=True, stop=True)
                dg_row = constp.tile([1, NBLK], F32)
                nc.vector.tensor_copy(dg_row[:], pf2[:])
                dg_tot = constp.tile([1, 1], F32)
                nc.vector.tensor_reduce(dg_tot[:], dg_row[:], axis=AX.X, op=ALU.add)
                nc.sync.dma_start(out_partial[0:1, 1:2], dg_tot[:])

                pf = psC.tile([1, 1], F32, tag="misc", name="fin_sp")
                nc.tensor.matmul(
                    pf[0:1, 0:1], ones_col[:], sp_st[:, 0:1], start=True, stop=True
                )
                sp_tot = constp.tile([1, 1], F32)
                nc.vector.tensor_copy(sp_tot[:], pf[0:1, 0:1])
                nc.sync.dma_start(out_partial[0:1, 0:1], sp_tot[:])

    nc.compile()
    _NC_CACHE["nc"] = nc
    return nc


def make_in_maps(emb_i, emb_j, labels):
    emb_i = np.ascontiguousarray(np.asarray(emb_i, dtype=np.float32))
    emb_j = np.ascontiguousarray(np.asarray(emb_j, dtype=np.float32))
    lab = np.asarray(labels).astype(np.int64)

    cnt = np.bincount(lab, minlength=C).astype(np.float64)
    sc = (-2.0 / cnt).astype(np.float32)
    ic2 = (1.0 / (cnt * cnt)).astype(np.float32)

    def colmat(v):
        # [C] -> [P, NCC]: value for class cc*P + p lands at [p, cc]
        return v.reshape(NCC, P).T

    aux = np.zeros((P, NAUX), dtype=np.float32)
    aux[:, 0:128] = np.arange(P, dtype=np.float32)[None, :]
    aux[:, 136:144] = (
        np.arange(P, dtype=np.float32)[:, None]
        + P * np.arange(NCC, dtype=np.float32)[None, :]
    )
    aux[:, 144:152] = colmat(sc)
    aux[:, 152:160] = colmat(ic2)

    in_maps = []
    for k in range(N_CORES):
        sel = np.nonzero((lab >= k * CW) & (lab < (k + 1) * CW))[0]
        assert len(sel) <= BL, f"bucket {k} overflow: {len(sel)}"
        ei = np.zeros((BL, D), dtype=np.float32)
        ei[: len(sel)] = emb_i[sel]
        ll = np.zeros((BL,), dtype=np.float32)
        ll[: len(sel)] = (lab[sel] - k * CW).astype(np.float32)
        aux_k = aux.copy()
        aux_k[:, 128:136] = ll.reshape(NB, P).T

        sl = slice(k * BL, (k + 1) * BL)
        lab_k = lab[sl].astype(np.float32)
        in_maps.append(
            {
                "emb_i": ei,
                "emb_jT": np.ascontiguousarray(emb_j[sl].T),
                "label_bcast": np.ascontiguousarray(
                    np.broadcast_to(lab_k[None, :], (P, BL))
                ),
                "aux": aux_k,
            }
        )
    return in_maps


def combine_partials(results):
    tot = 0.0
    for k in range(N_CORES):
        p = np.asarray(results[k]["out_partial"], dtype=np.float64)
        tot += p[0, 0] + p[0, 1]
    loss = (tot - 2.0 * B) / (B * C)
    return np.asarray(np.float32(loss))


def _numpy_fallback(emb_i, emb_j, labels):
    emb_i = np.asarray(emb_i, dtype=np.float64)
    emb_j = np.asarray(emb_j, dtype=np.float64)
    lab = np.asarray(labels).astype(np.int64)
    zi = emb_i / np.maximum(np.linalg.norm(emb_i, axis=1, keepdims=True), EPS)
    zj = emb_j / np.maximum(np.linalg.norm(emb_j, axis=1, keepdims=True), EPS)
    cnt = np.bincount(lab, minlength=C).astype(np.float64)
    seg = np.zeros((C, D))
    np.add.at(seg, lab, zi)
    proto = seg / cnt[:, None]
    d2 = (
        (zj * zj).sum(1)[:, None]
        + (proto * proto).sum(1)[None, :]
        - 2.0 * zj @ proto.T
    )
    sim = 2.0 - np.sqrt(np.maximum(d2, 0.0))
    match = (np.arange(C)[None, :] == lab[:, None]).astype(np.float64)
    sp = np.logaddexp(0.0, sim)
    loss = np.mean(sp - match * sim)
    return np.asarray(np.float32(loss))


def run(emb_i, emb_j, labels, **run_kwargs):
    nc = build_nc()
    in_maps = make_in_maps(emb_i, emb_j, labels)
    res = bass_utils.run_bass_kernel_spmd(
        nc, in_maps, core_ids=list(range(N_CORES)), **run_kwargs
    )
    return combine_partials(res.results), res


def kernel(emb_i, emb_j, labels):
    lab = np.asarray(labels).astype(np.int64)
    sizes = np.bincount(lab // CW, minlength=N_CORES)
    if sizes.max() > BL or np.bincount(lab, minlength=C).min() == 0:
        return _numpy_fallback(emb_i, emb_j, labels)
    loss, _ = run(emb_i, emb_j, labels)
    return loss
